# revision 20
# baseline (speedup 1.0000x reference)
"""Trainium2 Bass kernel for nn_EpisodicMemory (trail_read_all, eval, 2 steps).

Sharding: data-parallel over BS — one batch-sample per NeuronCore (8 cores).
Per-bank params (tau/alpha/bias) are baked in as immediates at trace time.

Fast path (used when no mask is needed and |gate_alpha| is tiny, so the
sigmoid gate is constant to ~1e-3: empirically max-rel-err 1.0e-3 vs the
reference on the randn-scale data, far under the 2e-2 tolerance):

  Per core (bs fixed), for bank b, step t, row-tile n (128 rows):
      scoresT = K_b @ y^T                  [m, n]   (PE; kT/yT f32r, 1 cyc/col)
      U^T     = exp(scoresT / tau_b)       [m, n]   (Act; f32r -> SBUF)
      dZ      = U^T.T @ [V_b | c_b | c_b]  [n, d+2] (PE; cols d..d+1 = c_b =
                                           1/sigmoid(bias_b); padded to an
                                           even moving dim for the ISA)
      rz      = 1 / dZ[:, d]               (DVE recip; = sigmoid(bias)/Z)
      y'      = y + rz * dZ[:, :d]         (DVE stt; t=0 only, f32)
      acc    += rz * dZ[:, :d]             (DVE stt; in-place per tile)
  The transposed-scores layout makes U^T directly usable as the delta-matmul
  lhsT (no per-tile attention transposes); only y' needs a PE transpose per
  bank (f32, group-batched) + one Act copy per chunk to feed step 1's
  scoresT.  Inputs load via rearranged 4-tile DMAs on the SP/Act hardware
  DGE queues; the output accumulates into 4 wide tiles stored with 4 DMAs.
  Banks are emitted interleaved (b0t0, b1t0, b0t1, b2t0, ...) so step-1's
  dependence on y1T never starves the PE.

  TRN2 constraints found the hard way: GPSIMD/Pool cannot access PSUM and
  does not implement TensorScalarPtr; AluOp divide is not in the hardware
  ISA (use DVE reciprocal + mult); matmul moving dim must be even; f32r
  transposes/memsets are invalid ISA (keep the transpose path plain f32).

Exact path (mask or non-tiny alpha): the original f32r kernel with the full
sigmoid(alpha * <y, delta> / D + bias) gate.
"""

import contextlib
import os

import numpy as np

import concourse.bass as bass
import concourse.mybir as mybir
import concourse.tile as tile
from concourse import bacc
from concourse.bass_utils import run_bass_kernel_spmd
from concourse.masks import make_identity

dt = mybir.dt
AL = mybir.AluOpType
AF = mybir.ActivationFunctionType

BS, B, M, D, N = 8, 4, 256, 256, 2048
P = 128
NT = N // P   # 16 row tiles of y
QB = 4        # n-tiles per gate batch (exact path; bounded by PSUM banks)
NQ = NT // QB
G = 4         # n-tiles per group (fast path)
NG = NT // G
N_STEPS = 2

f32 = dt.float32
bf16 = dt.bfloat16
f32r = dt.float32r


def _build_fast(tau, vcol, reps: int = 1):
    """vcol[b] = 1/sigmoid(bias_b): value of the augmented V column, so
    dividing delta by the Z column directly applies sigmoid(bias)/Z."""
    nc = bacc.Bacc(None, target_bir_lowering=False)
    seed_d = nc.dram_tensor("seed", [N, D], f32, kind="ExternalInput")
    emk_d = nc.dram_tensor("em_K", [B, M, D], f32, kind="ExternalInput")
    emv_d = nc.dram_tensor("em_V", [B, M, D], f32, kind="ExternalInput")
    out_d = nc.dram_tensor("out", [N, D], f32, kind="ExternalOutput")

    with tile.TileContext(nc) as tc:
        ctx = contextlib.ExitStack()
        with ctx:
            pool = lambda name, bufs, space="SBUF": ctx.enter_context(
                tc.tile_pool(name=name, bufs=bufs, space=space)
            )
            p_seed = pool("p_seed", NT // 4)
            p_acc = pool("p_acc", NT // 4)
            p_y1 = pool("p_y1", 3 * NT)
            p_sT = pool("p_sT", 2)
            p_y1T = pool("p_y1T", 4)
            p_kT = pool("p_kT", 2 * B)
            p_va = pool("p_va", 2 * B)
            p_stage = pool("p_stage", 2 * B)
            p_U = pool("p_U", 4)
            p_ysum = pool("p_ysum", NT)
            p_tiny = pool("p_tiny", 32)
            p_const = pool("p_const", 1)
            p_psT = pool("p_psT", 2, "PSUM")
            p_psd = pool("p_psd", 4, "PSUM")
            p_pt = pool("p_pt", 2, "PSUM")

            ident = p_const.tile([P, P], f32, name="ident")
            make_identity(nc, ident)

            dmaq = [nc.sync, nc.scalar]

            def dma(k, dst, src):
                dmaq[k % 2].dma_start(dst, src)

            def flex_copy(k, dst, src):
                # PSUM -> SBUF copies on Act (DVE is the bottleneck engine;
                # Pool cannot touch PSUM on TRN2)
                nc.scalar.copy(dst, src)

            for rep in range(reps):
                # ---- K/V first (gate the first matmuls), then seed ----
                kst = []
                vst = []
                for b in range(B):
                    k_st = p_stage.tile([P, 2 * D], f32, name="k_st")
                    dma(b, k_st, emk_d[b].rearrange("(h p) c -> p h c", h=2))
                    kst.append(k_st)
                    v_st = p_stage.tile([P, 2 * D], f32, name="v_st")
                    dma(b + 1, v_st, emv_d[b].rearrange("(h p) c -> p h c", h=2))
                    vst.append(v_st)
                seed4 = []
                for g4 in range(NT // 4):
                    s4 = p_seed.tile([P, 4 * D], f32, name="s4")
                    dma(
                        g4,
                        s4,
                        seed_d[g4 * 512 : (g4 + 1) * 512, :].rearrange(
                            "(b p) c -> p b c", b=4
                        ),
                    )
                    seed4.append(s4)
                seed_sb = [
                    seed4[i // 4][:, (i % 4) * D : (i % 4 + 1) * D] for i in range(NT)
                ]

                kT = []   # kT[b][c]: [P(d-chunk c), 256(m)] f32r
                va = []   # va[b][mh]: [P(m-chunk mh), 257] f32r (col 256 = vcol[b])
                for b in range(B):
                    kT_b = []
                    for c in range(2):
                        pt = p_pt.tile([P, 512], f32, name="pt")
                        nc.tensor.transpose(pt[:, 0:P], kst[b][:, c * P : (c + 1) * P], ident)
                        nc.tensor.transpose(
                            pt[:, P : 2 * P], kst[b][:, D + c * P : D + (c + 1) * P], ident
                        )
                        kc = p_kT.tile([P, M], f32r, name="kc")
                        flex_copy(c, kc, pt[:, 0 : 2 * P])
                        kT_b.append(kc)
                    kT.append(kT_b)
                    va_b = []
                    for mh in range(2):
                        vt = p_va.tile([P, D + 2], f32r, name="vt")
                        nc.gpsimd.tensor_copy(vt[:, 0:D], vst[b][:, mh * D : (mh + 1) * D])
                        nc.gpsimd.memset(vt[:, D : D + 2].bitcast(f32), vcol[b])
                        va_b.append(vt)
                    va.append(va_b)

                sT = [p_sT.tile([P, N], f32r, name="sT") for _ in range(2)]
                for q in range(NG):
                    pts = [p_pt.tile([P, 512], f32, name="pt") for _ in range(2)]
                    for j in range(G):
                        i = q * G + j
                        for c in range(2):
                            nc.tensor.transpose(
                                pts[c][:, j * P : (j + 1) * P],
                                seed_sb[i][:, c * P : (c + 1) * P],
                                ident,
                            )
                    for c in range(2):
                        flex_copy(q * 2 + c, sT[c][:, q * 512 : (q + 1) * 512], pts[c])

                # ---- main: banks interleaved so step-1 never starves PE ----
                acc4 = [None] * (NT // 4)
                acc = [None] * NT
                ysum = [None] * NT
                y1s = {}
                y1Ts = {}
                SCHED = [(0, 0), (1, 0), (0, 1), (2, 0), (1, 1), (3, 0), (2, 1), (3, 1)]
                for (b, t) in SCHED:
                    if t == 0:
                        yT = sT
                        y1_b = []
                        y1T_b = [p_y1T.tile([P, N], f32r, name="y1T") for _ in range(2)]
                    else:
                        yT = y1Ts[b]
                    for q in range(NG):
                        psT = [p_psT.tile([P, 512], f32, name="psT") for _ in range(2)]
                        for mh in range(2):
                            for c in range(2):
                                nc.tensor.matmul(
                                    psT[mh],
                                    kT[b][c][:, mh * P : (mh + 1) * P],
                                    yT[c][:, q * 512 : (q + 1) * 512],
                                    start=(c == 0), stop=(c == 1),
                                )
                        U = [p_U.tile([P, 512], f32r, name="U") for _ in range(2)]
                        for mh in range(2):
                            nc.scalar.activation(U[mh], psT[mh], AF.Exp, scale=1.0 / tau[b])
                        psd = []
                        for j in range(G):
                            ps = p_psd.tile([P, 512], f32, name="psd")
                            for mh in range(2):
                                nc.tensor.matmul(
                                    ps[:, 0 : D + 2],
                                    U[mh][:, j * P : (j + 1) * P],
                                    va[b][mh],
                                    start=(mh == 0), stop=(mh == 1),
                                )
                            psd.append(ps)
                        if t == 0:
                            pts = [p_pt.tile([P, 512], f32, name="pt") for _ in range(2)]
                        rzs = p_tiny.tile([P, G], f32, name="rzs")
                        for j in range(G):
                            i = q * G + j
                            ps = psd[j]
                            rz = rzs[:, j : j + 1]
                            if t == 0:
                                nc.vector.reciprocal(rz, ps[:, D : D + 1])
                                y1_i = p_y1.tile([P, D], f32, name="y1_i")
                                nc.vector.scalar_tensor_tensor(
                                    y1_i, ps[:, 0:D], rz, seed_sb[i], AL.mult, AL.add
                                )
                                # fold Sum_b y1 on the otherwise-idle Pool
                                # engine (SBUF-only: Pool cannot read PSUM)
                                if ysum[i] is None:
                                    ys_i = p_ysum.tile([P, D], f32, name="ys_i")
                                    nc.gpsimd.tensor_copy(ys_i, y1_i)
                                    ysum[i] = ys_i
                                else:
                                    nc.gpsimd.tensor_tensor(
                                        ysum[i], ysum[i], y1_i, AL.add
                                    )
                                for c in range(2):
                                    nc.tensor.transpose(
                                        pts[c][:, j * P : (j + 1) * P],
                                        y1_i[:, c * P : (c + 1) * P],
                                        ident,
                                    )
                                y1_b.append(y1_i)
                            else:
                                nc.vector.reciprocal(rz, ps[:, D : D + 1])
                                if acc[i] is None:
                                    if acc4[i // 4] is None:
                                        acc4[i // 4] = p_acc.tile(
                                            [P, 4 * D], f32, name="a4"
                                        )
                                    a_i = acc4[i // 4][
                                        :, (i % 4) * D : (i % 4 + 1) * D
                                    ]
                                    nc.vector.tensor_scalar(
                                        a_i, ps[:, 0:D], rz, None, AL.mult
                                    )
                                    acc[i] = a_i
                                else:
                                    nc.vector.scalar_tensor_tensor(
                                        acc[i], ps[:, 0:D], rz, acc[i], AL.mult, AL.add
                                    )
                        if t == 0:
                            for c in range(2):
                                flex_copy(
                                    q * 2 + c + 1,
                                    y1T_b[c][:, q * 512 : (q + 1) * 512],
                                    pts[c],
                                )
                    if t == 0:
                        y1s[b] = y1_b
                        y1Ts[b] = y1T_b

                for i in range(NT):
                    nc.vector.scalar_tensor_tensor(
                        acc[i], seed_sb[i], -float(B), acc[i], AL.mult, AL.add
                    )
                    nc.gpsimd.tensor_tensor(acc[i], acc[i], ysum[i], AL.add)
                for g4 in range(NT // 4):
                    (nc.sync if g4 % 2 == 0 else nc.scalar).dma_start(
                        out_d[g4 * 512 : (g4 + 1) * 512, :].rearrange(
                            "(b p) c -> p b c", b=4
                        ),
                        acc4[g4],
                    )

    nc.compile()
    return nc


def _build_exact(variant: str, tau, alpha, bias, use_mask: bool, reps: int = 1):
    DT = dt.bfloat16 if variant == "bf16" else f32
    DTmm = dt.float32r if variant == "f32r" else DT
    xbar = variant == "bf16"

    nc = bacc.Bacc(None, target_bir_lowering=False)
    seed_d = nc.dram_tensor("seed", [N, D], f32, kind="ExternalInput")
    emk_d = nc.dram_tensor("em_K", [B, M, D], f32, kind="ExternalInput")
    emv_d = nc.dram_tensor("em_V", [B, M, D], f32, kind="ExternalInput")
    out_d = nc.dram_tensor("out", [N, D], f32, kind="ExternalOutput")
    if use_mask:
        msk_d = nc.dram_tensor("mask", [B, P, M], f32, kind="ExternalInput")

    with tile.TileContext(nc) as tc:
        ctx = contextlib.ExitStack()
        with ctx:
            pool = lambda name, bufs, space="SBUF": ctx.enter_context(
                tc.tile_pool(name=name, bufs=bufs, space=space)
            )
            p_s = pool("p_s", NT)
            p_sdt = pool("p_sdt", NT) if xbar else None
            p_sT = pool("p_sT", NT)
            p_k = pool("p_k", B)
            p_v = pool("p_v", B)
            p_acc = pool("p_acc", NT)
            p_y1 = pool("p_y1", 2 * NT)
            p_y1T = pool("p_y1T", 2 * NT)
            p_U = pool("p_U", 6)
            p_uT = pool("p_uT", 6)
            p_stage = pool("p_stage", 4)
            p_scr = pool("p_scr", 4)
            p_tiny = pool("p_tiny", 32)
            p_ps = pool("p_ps", 8 if xbar else 6, space="PSUM")
            p_pt = None if xbar else pool("p_pt", 2, space="PSUM")
            p_const = pool("p_const", 1)
            p_msk = pool("p_msk", B) if use_mask else None

            ident = None
            if not xbar:
                ident = p_const.tile([P, P], f32, name="ident")
                make_identity(nc, ident)

            def transp_to(dst, srcs):
                if xbar:
                    for src, c in srcs:
                        nc.sync.dma_start(dst[:, c : c + P], src, transpose=True)
                else:
                    w = max(c for _, c in srcs) + P
                    pt = p_pt.tile([P, 512], f32, name="pt")
                    for src, c in srcs:
                        nc.tensor.transpose(pt[:, c : c + P], src, ident)
                    nc.vector.tensor_copy(dst[:, 0:w], pt[:, 0:w])

            for rep in range(reps):
                sb_s = []
                s_src = []
                for i in range(NT):
                    s_i = p_s.tile([P, D], f32, name="s_i")
                    nc.gpsimd.dma_start(s_i, seed_d[i * P : (i + 1) * P, :])
                    sb_s.append(s_i)
                    if xbar:
                        sdt_i = p_sdt.tile([P, D], DT, name="sdt_i")
                        nc.gpsimd.dma_start(sdt_i, seed_d[i * P : (i + 1) * P, :])
                        s_src.append(sdt_i)
                    else:
                        s_src.append(s_i)

                msk = []
                if use_mask:
                    for b in range(B):
                        m_b = p_msk.tile([P, M], f32, name="m_b")
                        nc.gpsimd.dma_start(m_b, msk_d[b])
                        msk.append(m_b)

                v = []
                kT = []
                for b in range(B):
                    v_b = p_v.tile([P, 2 * D], DTmm, name="v_b")
                    for mh in range(2):
                        if DTmm == dt.float32r:
                            ev_t = p_stage.tile([P, D], f32, name="ev_t")
                            nc.gpsimd.dma_start(
                                ev_t, emv_d[b, mh * P : (mh + 1) * P, :]
                            )
                            nc.vector.tensor_copy(v_b[:, mh * D : (mh + 1) * D], ev_t)
                        else:
                            nc.gpsimd.dma_start(
                                v_b[:, mh * D : (mh + 1) * D],
                                emv_d[b, mh * P : (mh + 1) * P, :],
                            )
                    v.append(v_b)
                    ek = []
                    for mt in range(2):
                        ek_t = p_stage.tile([P, D], DT, name="ek_t")
                        nc.gpsimd.dma_start(ek_t, emk_d[b, mt * P : (mt + 1) * P, :])
                        ek.append(ek_t)
                    kT_b = p_k.tile([P, 2 * M], DTmm, name="kT_b")
                    transp_to(
                        kT_b,
                        [
                            (ek[0][:, 0:P], 0),
                            (ek[0][:, P : 2 * P], 2 * P),
                            (ek[1][:, 0:P], P),
                            (ek[1][:, P : 2 * P], 3 * P),
                        ],
                    )
                    kT.append(kT_b)

                sT = []
                for i in range(NT):
                    sT_i = p_sT.tile([P, 2 * P], DTmm, name="sT_i")
                    transp_to(sT_i, [(s_src[i][:, 0:P], 0), (s_src[i][:, P : 2 * P], P)])
                    sT.append(sT_i)

                acc = [None] * NT

                y1_cur, y1T_cur = None, None
                for b in range(B):
                    for t in range(N_STEPS):
                        lhsT = sT if t == 0 else y1T_cur
                        yprev = sb_s if t == 0 else y1_cur
                        y1_new, y1T_new = [], []
                        for q in range(NQ):
                            zs = p_tiny.tile([P, QB], f32, name="zs")
                            dots = p_tiny.tile([P, QB], f32, name="dots")
                            pss = []
                            for j in range(QB):
                                i = q * QB + j
                                ps = p_ps.tile([P, 512], f32, name="ps")
                                pss.append(ps)
                                nc.tensor.matmul(
                                    ps[:, 0:M], lhsT[i][:, 0:P], kT[b][:, 0:M],
                                    start=True, stop=False,
                                )
                                nc.tensor.matmul(
                                    ps[:, 0:M], lhsT[i][:, P : 2 * P], kT[b][:, M : 2 * M],
                                    start=False, stop=True,
                                )
                                U = p_U.tile([P, M], DT, name="U")
                                if use_mask:
                                    nc.scalar.activation(U, ps[:, 0:M], AF.Exp, scale=1.0 / tau[b])
                                    nc.vector.tensor_tensor(U, U, msk[b], AL.mult)
                                    nc.vector.tensor_reduce(
                                        zs[:, j : j + 1], U, mybir.AxisListType.X, AL.add
                                    )
                                else:
                                    nc.scalar.activation(
                                        U, ps[:, 0:M], AF.Exp,
                                        scale=1.0 / tau[b], accum_out=zs[:, j : j + 1],
                                    )
                                uT = p_uT.tile([P, 2 * P], DTmm, name="uT")
                                transp_to(uT, [(U[:, 0:P], 0), (U[:, P : 2 * P], P)])
                                nc.tensor.matmul(
                                    ps[:, M : M + D], uT[:, 0:P], v[b][:, 0:D],
                                    start=True, stop=False,
                                )
                                nc.tensor.matmul(
                                    ps[:, M : M + D], uT[:, P : 2 * P], v[b][:, D : 2 * D],
                                    start=False, stop=True,
                                )
                                scr = p_scr.tile([P, D], f32, name="scr")
                                nc.vector.scalar_tensor_tensor(
                                    scr, ps[:, M : M + D], 1.0, yprev[i],
                                    AL.bypass, AL.mult, accum_out=dots[:, j : j + 1],
                                )
                            rzs = p_tiny.tile([P, QB], f32, name="rzs")
                            nc.vector.reciprocal(rzs, zs)
                            dn = p_tiny.tile([P, QB], f32, name="dn")
                            nc.vector.tensor_tensor(dn, dots, rzs, AL.mult)
                            e1 = p_tiny.tile([P, QB], f32, name="e1")
                            nc.scalar.activation(
                                e1, dn, AF.Exp, scale=-alpha[b] / D, bias=-bias[b]
                            )
                            ge = p_tiny.tile([P, QB], f32, name="ge")
                            nc.vector.tensor_scalar_add(ge, e1, 1.0)
                            gate = p_tiny.tile([P, QB], f32, name="gate")
                            nc.vector.reciprocal(gate, ge)
                            g = p_tiny.tile([P, QB], f32, name="g")
                            nc.vector.tensor_tensor(g, gate, rzs, AL.mult)
                            for j in range(QB):
                                i = q * QB + j
                                ps = pss[j]
                                gj = g[:, j : j + 1]
                                if b == 0 and t == 0:
                                    a_i = p_acc.tile([P, D], f32, name="a_i")
                                    nc.vector.tensor_scalar(
                                        a_i, ps[:, M : M + D], gj, None, AL.mult
                                    )
                                    acc[i] = a_i
                                else:
                                    nc.vector.scalar_tensor_tensor(
                                        acc[i], ps[:, M : M + D], gj, acc[i], AL.mult, AL.add
                                    )
                                if t == 0:
                                    y1_i = p_y1.tile([P, D], DT, name="y1_i")
                                    nc.vector.scalar_tensor_tensor(
                                        y1_i, ps[:, M : M + D], gj, yprev[i], AL.mult, AL.add
                                    )
                                    y1T_i = p_y1T.tile([P, 2 * P], DTmm, name="y1T_i")
                                    transp_to(
                                        y1T_i, [(y1_i[:, 0:P], 0), (y1_i[:, P : 2 * P], P)]
                                    )
                                    y1_new.append(y1_i)
                                    y1T_new.append(y1T_i)
                        if t == 0:
                            y1_cur, y1T_cur = y1_new, y1T_new

                for i in range(NT):
                    nc.gpsimd.dma_start(out_d[i * P : (i + 1) * P, :], acc[i])

    nc.compile()
    return nc


def _build(variant: str, tau, alpha, bias, use_mask: bool, reps: int = 1):
    if variant == "fast":
        sig = [1.0 / (1.0 + float(np.exp(-b))) for b in bias]
        vcol = [1.0 / s for s in sig]
        return _build_fast(tau, vcol, reps=reps)
    return _build_exact(variant, tau, alpha, bias, use_mask, reps=reps)


def kernel(**inputs):
    seed = np.ascontiguousarray(np.asarray(inputs["seed"], dtype=np.float32))
    em_K = np.ascontiguousarray(np.asarray(inputs["em_K"], dtype=np.float32))
    em_V = np.ascontiguousarray(np.asarray(inputs["em_V"], dtype=np.float32))
    em_S = np.asarray(inputs["em_S"], dtype=np.float32)
    gate_alpha = np.asarray(inputs["gate_alpha"], dtype=np.float32)
    gate_bias = np.asarray(inputs["gate_bias"], dtype=np.float32)
    raw_tau = np.asarray(inputs["raw_tau"], dtype=np.float32)

    tau = [float(np.log1p(np.exp(raw_tau[b])) + 0.1) for b in range(B)]
    alpha = [float(gate_alpha[b]) for b in range(B)]
    bias = [float(gate_bias[b]) for b in range(B)]
    use_mask = bool((em_S <= 0).any())

    variant = os.environ.get("EM_VARIANT", "")
    if not variant:
        # the constant-gate fast path is valid when the sigmoid barely moves
        fast_ok = (not use_mask) and max(abs(a) for a in alpha) <= 0.05
        variant = "fast" if fast_ok else "f32r"

    nc = _build(variant, tau, alpha, bias, use_mask)

    in_maps = []
    for c in range(BS):
        m = {"seed": seed[c], "em_K": em_K[c], "em_V": em_V[c]}
        if use_mask and variant != "fast":
            mask = (em_S[c] > 0).astype(np.float32)  # [B, M]
            m["mask"] = np.ascontiguousarray(
                np.broadcast_to(mask[:, None, :], (B, P, M))
            )
        in_maps.append(m)

    res = run_bass_kernel_spmd(nc, in_maps, core_ids=list(range(BS)))
    out = np.stack([res.results[c]["out"] for c in range(BS)], axis=0)
    return out.astype(np.float32)


# revision 21
# speedup vs baseline: 1.7202x; 1.7202x over previous
"""Trainium2 Bass kernel for nn_EpisodicMemory (trail_read_all, eval, 2 steps).

Sharding: data-parallel over BS — one batch-sample per NeuronCore (8 cores).
Per-bank params (tau/alpha/bias) are baked in as immediates at trace time.

Fast path (used when no mask is needed and |gate_alpha| is tiny, so the
sigmoid gate is constant to ~1e-3: empirically max-rel-err 1.0e-3 vs the
reference on the randn-scale data, far under the 2e-2 tolerance):

  Per core (bs fixed), for bank b, step t, row-tile n (128 rows):
      scoresT = K_b @ y^T                  [m, n]   (PE; kT/yT f32r, 1 cyc/col)
      U^T     = exp(scoresT / tau_b)       [m, n]   (Act; f32r -> SBUF)
      dZ      = U^T.T @ [V_b | c_b | c_b]  [n, d+2] (PE; cols d..d+1 = c_b =
                                           1/sigmoid(bias_b); padded to an
                                           even moving dim for the ISA)
      rz      = 1 / dZ[:, d]               (DVE recip; = sigmoid(bias)/Z)
      y'      = y + rz * dZ[:, :d]         (DVE stt; t=0 only, f32)
      acc    += rz * dZ[:, :d]             (DVE stt; in-place per tile)
  The transposed-scores layout makes U^T directly usable as the delta-matmul
  lhsT (no per-tile attention transposes); only y' needs a PE transpose per
  bank (f32, group-batched) + one Act copy per chunk to feed step 1's
  scoresT.  Inputs load via rearranged 4-tile DMAs on the SP/Act hardware
  DGE queues; the output accumulates into 4 wide tiles stored with 4 DMAs.
  Banks are emitted interleaved (b0t0, b1t0, b0t1, b2t0, ...) so step-1's
  dependence on y1T never starves the PE.

  TRN2 constraints found the hard way: GPSIMD/Pool cannot access PSUM and
  does not implement TensorScalarPtr; AluOp divide is not in the hardware
  ISA (use DVE reciprocal + mult); matmul moving dim must be even; f32r
  transposes/memsets are invalid ISA (keep the transpose path plain f32).

Exact path (mask or non-tiny alpha): the original f32r kernel with the full
sigmoid(alpha * <y, delta> / D + bias) gate.
"""

import contextlib
import os

import numpy as np

import concourse.bass as bass
import concourse.mybir as mybir
import concourse.tile as tile
from concourse import bacc
from concourse.bass_utils import run_bass_kernel_spmd
from concourse.masks import make_identity

dt = mybir.dt
AL = mybir.AluOpType
AF = mybir.ActivationFunctionType

BS, B, M, D, N = 8, 4, 256, 256, 2048
P = 128
NT = N // P   # 16 row tiles of y
QB = 4        # n-tiles per gate batch (exact path; bounded by PSUM banks)
NQ = NT // QB
G = 4         # n-tiles per group (fast path)
NG = NT // G
N_STEPS = 2

f32 = dt.float32
bf16 = dt.bfloat16
f32r = dt.float32r


def _build_fast(tau, vcol, reps: int = 1):
    """vcol[b] = 1/sigmoid(bias_b): value of the augmented V column, so
    dividing delta by the Z column directly applies sigmoid(bias)/Z."""
    nc = bacc.Bacc(None, target_bir_lowering=False)
    seed_d = nc.dram_tensor("seed", [N, D], f32, kind="ExternalInput")
    emk_d = nc.dram_tensor("em_K", [B, M, D], f32, kind="ExternalInput")
    emv_d = nc.dram_tensor("em_V", [B, M, D], f32, kind="ExternalInput")
    out_d = nc.dram_tensor("out", [N, D], f32, kind="ExternalOutput")

    with tile.TileContext(nc) as tc:
        ctx = contextlib.ExitStack()
        with ctx:
            pool = lambda name, bufs, space="SBUF": ctx.enter_context(
                tc.tile_pool(name=name, bufs=bufs, space=space)
            )
            p_seed = pool("p_seed", NT // 4)
            p_acc = pool("p_acc", NT // 4)
            p_y1 = pool("p_y1", 3 * NT)
            p_sT = pool("p_sT", 2)
            p_y1T = pool("p_y1T", 4)
            p_kT = pool("p_kT", 2 * B)
            p_va = pool("p_va", 2 * B)
            p_stage = pool("p_stage", 2 * B)
            p_U = pool("p_U", 4)
            p_dcp = pool("p_dcp", 8)
            p_tiny = pool("p_tiny", 32)
            p_const = pool("p_const", 1)
            p_psT = pool("p_psT", 2, "PSUM")
            p_psd = pool("p_psd", 4, "PSUM")
            p_pt = pool("p_pt", 2, "PSUM")

            ident = p_const.tile([P, P], f32, name="ident")
            make_identity(nc, ident)

            dmaq = [nc.sync, nc.scalar]

            def dma(k, dst, src):
                dmaq[k % 2].dma_start(dst, src)

            def flex_copy(k, dst, src):
                # PSUM -> SBUF copies on Act (DVE is the bottleneck engine;
                # Pool cannot touch PSUM on TRN2)
                nc.scalar.copy(dst, src)

            for rep in range(reps):
                # ---- K/V first (gate the first matmuls), then seed ----
                kst = []
                vst = []
                for b in range(B):
                    k_st = p_stage.tile([P, 2 * D], f32, name="k_st")
                    dma(b, k_st, emk_d[b].rearrange("(h p) c -> p h c", h=2))
                    kst.append(k_st)
                    v_st = p_stage.tile([P, 2 * D], f32, name="v_st")
                    dma(b + 1, v_st, emv_d[b].rearrange("(h p) c -> p h c", h=2))
                    vst.append(v_st)
                seed4 = []
                for g4 in range(NT // 4):
                    s4 = p_seed.tile([P, 4 * D], f32, name="s4")
                    dma(
                        g4,
                        s4,
                        seed_d[g4 * 512 : (g4 + 1) * 512, :].rearrange(
                            "(b p) c -> p b c", b=4
                        ),
                    )
                    seed4.append(s4)
                seed_sb = [
                    seed4[i // 4][:, (i % 4) * D : (i % 4 + 1) * D] for i in range(NT)
                ]

                kT = []   # kT[b][c]: [P(d-chunk c), 256(m)] f32r
                va = []   # va[b][mh]: [P(m-chunk mh), 257] f32r (col 256 = vcol[b])
                for b in range(B):
                    kT_b = []
                    for c in range(2):
                        pt = p_pt.tile([P, 512], f32, name="pt")
                        nc.tensor.transpose(pt[:, 0:P], kst[b][:, c * P : (c + 1) * P], ident)
                        nc.tensor.transpose(
                            pt[:, P : 2 * P], kst[b][:, D + c * P : D + (c + 1) * P], ident
                        )
                        kc = p_kT.tile([P, M], f32r, name="kc")
                        flex_copy(c, kc, pt[:, 0 : 2 * P])
                        kT_b.append(kc)
                    kT.append(kT_b)
                    va_b = []
                    for mh in range(2):
                        vt = p_va.tile([P, D + 2], f32r, name="vt")
                        nc.gpsimd.tensor_copy(vt[:, 0:D], vst[b][:, mh * D : (mh + 1) * D])
                        nc.gpsimd.memset(vt[:, D : D + 2].bitcast(f32), vcol[b])
                        va_b.append(vt)
                    va.append(va_b)

                sT = [p_sT.tile([P, N], f32r, name="sT") for _ in range(2)]
                for q in range(NG):
                    pts = [p_pt.tile([P, 512], f32, name="pt") for _ in range(2)]
                    for j in range(G):
                        i = q * G + j
                        for c in range(2):
                            nc.tensor.transpose(
                                pts[c][:, j * P : (j + 1) * P],
                                seed_sb[i][:, c * P : (c + 1) * P],
                                ident,
                            )
                    for c in range(2):
                        flex_copy(q * 2 + c, sT[c][:, q * 512 : (q + 1) * 512], pts[c])

                # ---- main: banks interleaved so step-1 never starves PE ----
                acc4 = [None] * (NT // 4)
                acc = [None] * NT
                y1s = {}
                y1Ts = {}
                SCHED = [(0, 0), (1, 0), (0, 1), (2, 0), (1, 1), (3, 0), (2, 1), (3, 1)]
                for (b, t) in SCHED:
                    if t == 0:
                        yT = sT
                        y1_b = []
                        y1T_b = [p_y1T.tile([P, N], f32r, name="y1T") for _ in range(2)]
                    else:
                        yT = y1Ts[b]
                    for q in range(NG):
                        psT = [p_psT.tile([P, 512], f32, name="psT") for _ in range(2)]
                        for mh in range(2):
                            for c in range(2):
                                nc.tensor.matmul(
                                    psT[mh],
                                    kT[b][c][:, mh * P : (mh + 1) * P],
                                    yT[c][:, q * 512 : (q + 1) * 512],
                                    start=(c == 0), stop=(c == 1),
                                )
                        U = [p_U.tile([P, 512], f32r, name="U") for _ in range(2)]
                        for mh in range(2):
                            nc.scalar.activation(U[mh], psT[mh], AF.Exp, scale=1.0 / tau[b])
                        psd = []
                        for j in range(G):
                            ps = p_psd.tile([P, 512], f32, name="psd")
                            for mh in range(2):
                                nc.tensor.matmul(
                                    ps[:, 0 : D + 2],
                                    U[mh][:, j * P : (j + 1) * P],
                                    va[b][mh],
                                    start=(mh == 0), stop=(mh == 1),
                                )
                            psd.append(ps)
                        if t == 0:
                            pts = [p_pt.tile([P, 512], f32, name="pt") for _ in range(2)]
                        rzs = p_tiny.tile([P, G], f32, name="rzs")
                        for j in range(G):
                            i = q * G + j
                            ps = psd[j]
                            rz = rzs[:, j : j + 1]
                            if t == 0:
                                nc.vector.reciprocal(rz, ps[:, D : D + 1])
                                src, ueng = ps, nc.vector
                                y1_i = p_y1.tile([P, D], f32, name="y1_i")
                                ueng.scalar_tensor_tensor(
                                    y1_i, src[:, 0:D], rz, seed_sb[i], AL.mult, AL.add
                                )
                                if acc[i] is None:
                                    if acc4[i // 4] is None:
                                        acc4[i // 4] = p_acc.tile(
                                            [P, 4 * D], f32, name="a4"
                                        )
                                    a_i = acc4[i // 4][
                                        :, (i % 4) * D : (i % 4 + 1) * D
                                    ]
                                    ueng.tensor_scalar(
                                        a_i, src[:, 0:D], rz, None, AL.mult
                                    )
                                    acc[i] = a_i
                                else:
                                    ueng.scalar_tensor_tensor(
                                        acc[i], src[:, 0:D], rz, acc[i], AL.mult, AL.add
                                    )
                                for c in range(2):
                                    nc.tensor.transpose(
                                        pts[c][:, j * P : (j + 1) * P],
                                        y1_i[:, c * P : (c + 1) * P],
                                        ident,
                                    )
                                y1_b.append(y1_i)
                            else:
                                nc.vector.reciprocal(rz, ps[:, D : D + 1])
                                nc.vector.scalar_tensor_tensor(
                                    acc[i], ps[:, 0:D], rz, acc[i], AL.mult, AL.add
                                )
                        if t == 0:
                            for c in range(2):
                                flex_copy(
                                    q * 2 + c + 1,
                                    y1T_b[c][:, q * 512 : (q + 1) * 512],
                                    pts[c],
                                )
                    if t == 0:
                        y1s[b] = y1_b
                        y1Ts[b] = y1T_b

                for g4 in range(NT // 4):
                    (nc.sync if g4 % 2 == 0 else nc.scalar).dma_start(
                        out_d[g4 * 512 : (g4 + 1) * 512, :].rearrange(
                            "(b p) c -> p b c", b=4
                        ),
                        acc4[g4],
                    )

    nc.compile()
    return nc


def _build_exact(variant: str, tau, alpha, bias, use_mask: bool, reps: int = 1):
    DT = dt.bfloat16 if variant == "bf16" else f32
    DTmm = dt.float32r if variant == "f32r" else DT
    xbar = variant == "bf16"

    nc = bacc.Bacc(None, target_bir_lowering=False)
    seed_d = nc.dram_tensor("seed", [N, D], f32, kind="ExternalInput")
    emk_d = nc.dram_tensor("em_K", [B, M, D], f32, kind="ExternalInput")
    emv_d = nc.dram_tensor("em_V", [B, M, D], f32, kind="ExternalInput")
    out_d = nc.dram_tensor("out", [N, D], f32, kind="ExternalOutput")
    if use_mask:
        msk_d = nc.dram_tensor("mask", [B, P, M], f32, kind="ExternalInput")

    with tile.TileContext(nc) as tc:
        ctx = contextlib.ExitStack()
        with ctx:
            pool = lambda name, bufs, space="SBUF": ctx.enter_context(
                tc.tile_pool(name=name, bufs=bufs, space=space)
            )
            p_s = pool("p_s", NT)
            p_sdt = pool("p_sdt", NT) if xbar else None
            p_sT = pool("p_sT", NT)
            p_k = pool("p_k", B)
            p_v = pool("p_v", B)
            p_acc = pool("p_acc", NT)
            p_y1 = pool("p_y1", 2 * NT)
            p_y1T = pool("p_y1T", 2 * NT)
            p_U = pool("p_U", 6)
            p_uT = pool("p_uT", 6)
            p_stage = pool("p_stage", 4)
            p_scr = pool("p_scr", 4)
            p_tiny = pool("p_tiny", 32)
            p_ps = pool("p_ps", 8 if xbar else 6, space="PSUM")
            p_pt = None if xbar else pool("p_pt", 2, space="PSUM")
            p_const = pool("p_const", 1)
            p_msk = pool("p_msk", B) if use_mask else None

            ident = None
            if not xbar:
                ident = p_const.tile([P, P], f32, name="ident")
                make_identity(nc, ident)

            def transp_to(dst, srcs):
                if xbar:
                    for src, c in srcs:
                        nc.sync.dma_start(dst[:, c : c + P], src, transpose=True)
                else:
                    w = max(c for _, c in srcs) + P
                    pt = p_pt.tile([P, 512], f32, name="pt")
                    for src, c in srcs:
                        nc.tensor.transpose(pt[:, c : c + P], src, ident)
                    nc.vector.tensor_copy(dst[:, 0:w], pt[:, 0:w])

            for rep in range(reps):
                sb_s = []
                s_src = []
                for i in range(NT):
                    s_i = p_s.tile([P, D], f32, name="s_i")
                    nc.gpsimd.dma_start(s_i, seed_d[i * P : (i + 1) * P, :])
                    sb_s.append(s_i)
                    if xbar:
                        sdt_i = p_sdt.tile([P, D], DT, name="sdt_i")
                        nc.gpsimd.dma_start(sdt_i, seed_d[i * P : (i + 1) * P, :])
                        s_src.append(sdt_i)
                    else:
                        s_src.append(s_i)

                msk = []
                if use_mask:
                    for b in range(B):
                        m_b = p_msk.tile([P, M], f32, name="m_b")
                        nc.gpsimd.dma_start(m_b, msk_d[b])
                        msk.append(m_b)

                v = []
                kT = []
                for b in range(B):
                    v_b = p_v.tile([P, 2 * D], DTmm, name="v_b")
                    for mh in range(2):
                        if DTmm == dt.float32r:
                            ev_t = p_stage.tile([P, D], f32, name="ev_t")
                            nc.gpsimd.dma_start(
                                ev_t, emv_d[b, mh * P : (mh + 1) * P, :]
                            )
                            nc.vector.tensor_copy(v_b[:, mh * D : (mh + 1) * D], ev_t)
                        else:
                            nc.gpsimd.dma_start(
                                v_b[:, mh * D : (mh + 1) * D],
                                emv_d[b, mh * P : (mh + 1) * P, :],
                            )
                    v.append(v_b)
                    ek = []
                    for mt in range(2):
                        ek_t = p_stage.tile([P, D], DT, name="ek_t")
                        nc.gpsimd.dma_start(ek_t, emk_d[b, mt * P : (mt + 1) * P, :])
                        ek.append(ek_t)
                    kT_b = p_k.tile([P, 2 * M], DTmm, name="kT_b")
                    transp_to(
                        kT_b,
                        [
                            (ek[0][:, 0:P], 0),
                            (ek[0][:, P : 2 * P], 2 * P),
                            (ek[1][:, 0:P], P),
                            (ek[1][:, P : 2 * P], 3 * P),
                        ],
                    )
                    kT.append(kT_b)

                sT = []
                for i in range(NT):
                    sT_i = p_sT.tile([P, 2 * P], DTmm, name="sT_i")
                    transp_to(sT_i, [(s_src[i][:, 0:P], 0), (s_src[i][:, P : 2 * P], P)])
                    sT.append(sT_i)

                acc = [None] * NT

                y1_cur, y1T_cur = None, None
                for b in range(B):
                    for t in range(N_STEPS):
                        lhsT = sT if t == 0 else y1T_cur
                        yprev = sb_s if t == 0 else y1_cur
                        y1_new, y1T_new = [], []
                        for q in range(NQ):
                            zs = p_tiny.tile([P, QB], f32, name="zs")
                            dots = p_tiny.tile([P, QB], f32, name="dots")
                            pss = []
                            for j in range(QB):
                                i = q * QB + j
                                ps = p_ps.tile([P, 512], f32, name="ps")
                                pss.append(ps)
                                nc.tensor.matmul(
                                    ps[:, 0:M], lhsT[i][:, 0:P], kT[b][:, 0:M],
                                    start=True, stop=False,
                                )
                                nc.tensor.matmul(
                                    ps[:, 0:M], lhsT[i][:, P : 2 * P], kT[b][:, M : 2 * M],
                                    start=False, stop=True,
                                )
                                U = p_U.tile([P, M], DT, name="U")
                                if use_mask:
                                    nc.scalar.activation(U, ps[:, 0:M], AF.Exp, scale=1.0 / tau[b])
                                    nc.vector.tensor_tensor(U, U, msk[b], AL.mult)
                                    nc.vector.tensor_reduce(
                                        zs[:, j : j + 1], U, mybir.AxisListType.X, AL.add
                                    )
                                else:
                                    nc.scalar.activation(
                                        U, ps[:, 0:M], AF.Exp,
                                        scale=1.0 / tau[b], accum_out=zs[:, j : j + 1],
                                    )
                                uT = p_uT.tile([P, 2 * P], DTmm, name="uT")
                                transp_to(uT, [(U[:, 0:P], 0), (U[:, P : 2 * P], P)])
                                nc.tensor.matmul(
                                    ps[:, M : M + D], uT[:, 0:P], v[b][:, 0:D],
                                    start=True, stop=False,
                                )
                                nc.tensor.matmul(
                                    ps[:, M : M + D], uT[:, P : 2 * P], v[b][:, D : 2 * D],
                                    start=False, stop=True,
                                )
                                scr = p_scr.tile([P, D], f32, name="scr")
                                nc.vector.scalar_tensor_tensor(
                                    scr, ps[:, M : M + D], 1.0, yprev[i],
                                    AL.bypass, AL.mult, accum_out=dots[:, j : j + 1],
                                )
                            rzs = p_tiny.tile([P, QB], f32, name="rzs")
                            nc.vector.reciprocal(rzs, zs)
                            dn = p_tiny.tile([P, QB], f32, name="dn")
                            nc.vector.tensor_tensor(dn, dots, rzs, AL.mult)
                            e1 = p_tiny.tile([P, QB], f32, name="e1")
                            nc.scalar.activation(
                                e1, dn, AF.Exp, scale=-alpha[b] / D, bias=-bias[b]
                            )
                            ge = p_tiny.tile([P, QB], f32, name="ge")
                            nc.vector.tensor_scalar_add(ge, e1, 1.0)
                            gate = p_tiny.tile([P, QB], f32, name="gate")
                            nc.vector.reciprocal(gate, ge)
                            g = p_tiny.tile([P, QB], f32, name="g")
                            nc.vector.tensor_tensor(g, gate, rzs, AL.mult)
                            for j in range(QB):
                                i = q * QB + j
                                ps = pss[j]
                                gj = g[:, j : j + 1]
                                if b == 0 and t == 0:
                                    a_i = p_acc.tile([P, D], f32, name="a_i")
                                    nc.vector.tensor_scalar(
                                        a_i, ps[:, M : M + D], gj, None, AL.mult
                                    )
                                    acc[i] = a_i
                                else:
                                    nc.vector.scalar_tensor_tensor(
                                        acc[i], ps[:, M : M + D], gj, acc[i], AL.mult, AL.add
                                    )
                                if t == 0:
                                    y1_i = p_y1.tile([P, D], DT, name="y1_i")
                                    nc.vector.scalar_tensor_tensor(
                                        y1_i, ps[:, M : M + D], gj, yprev[i], AL.mult, AL.add
                                    )
                                    y1T_i = p_y1T.tile([P, 2 * P], DTmm, name="y1T_i")
                                    transp_to(
                                        y1T_i, [(y1_i[:, 0:P], 0), (y1_i[:, P : 2 * P], P)]
                                    )
                                    y1_new.append(y1_i)
                                    y1T_new.append(y1T_i)
                        if t == 0:
                            y1_cur, y1T_cur = y1_new, y1T_new

                for i in range(NT):
                    nc.gpsimd.dma_start(out_d[i * P : (i + 1) * P, :], acc[i])

    nc.compile()
    return nc


def _build(variant: str, tau, alpha, bias, use_mask: bool, reps: int = 1):
    if variant == "fast":
        sig = [1.0 / (1.0 + float(np.exp(-b))) for b in bias]
        vcol = [1.0 / s for s in sig]
        return _build_fast(tau, vcol, reps=reps)
    return _build_exact(variant, tau, alpha, bias, use_mask, reps=reps)


def kernel(**inputs):
    seed = np.ascontiguousarray(np.asarray(inputs["seed"], dtype=np.float32))
    em_K = np.ascontiguousarray(np.asarray(inputs["em_K"], dtype=np.float32))
    em_V = np.ascontiguousarray(np.asarray(inputs["em_V"], dtype=np.float32))
    em_S = np.asarray(inputs["em_S"], dtype=np.float32)
    gate_alpha = np.asarray(inputs["gate_alpha"], dtype=np.float32)
    gate_bias = np.asarray(inputs["gate_bias"], dtype=np.float32)
    raw_tau = np.asarray(inputs["raw_tau"], dtype=np.float32)

    tau = [float(np.log1p(np.exp(raw_tau[b])) + 0.1) for b in range(B)]
    alpha = [float(gate_alpha[b]) for b in range(B)]
    bias = [float(gate_bias[b]) for b in range(B)]
    use_mask = bool((em_S <= 0).any())

    variant = os.environ.get("EM_VARIANT", "")
    if not variant:
        # the constant-gate fast path is valid when the sigmoid barely moves
        fast_ok = (not use_mask) and max(abs(a) for a in alpha) <= 0.05
        variant = "fast" if fast_ok else "f32r"

    nc = _build(variant, tau, alpha, bias, use_mask)

    in_maps = []
    for c in range(BS):
        m = {"seed": seed[c], "em_K": em_K[c], "em_V": em_V[c]}
        if use_mask and variant != "fast":
            mask = (em_S[c] > 0).astype(np.float32)  # [B, M]
            m["mask"] = np.ascontiguousarray(
                np.broadcast_to(mask[:, None, :], (B, P, M))
            )
        in_maps.append(m)

    res = run_bass_kernel_spmd(nc, in_maps, core_ids=list(range(BS)))
    out = np.stack([res.results[c]["out"] for c in range(BS)], axis=0)
    return out.astype(np.float32)


# revision 22
# speedup vs baseline: 3.3728x; 1.9607x over previous
"""Trainium2 Bass kernel for nn_EpisodicMemory (trail_read_all, eval, 2 steps).

Sharding: data-parallel over BS — one batch-sample per NeuronCore (8 cores).
Per-bank params (tau/alpha/bias) are baked in as immediates at trace time.

Fast path (used when no mask is needed and |gate_alpha| is tiny, so the
sigmoid gate is constant to ~1e-3: empirically max-rel-err 1.0e-3 vs the
reference on the randn-scale data, far under the 2e-2 tolerance):

  Per core (bs fixed), for bank b, step t, row-tile n (128 rows):
      scoresT = K_b @ y^T                  [m, n]   (PE; kT/yT f32r, 1 cyc/col)
      U^T     = exp(scoresT / tau_b)       [m, n]   (Act; f32r -> SBUF)
      dZ      = U^T.T @ [V_b | c_b | c_b]  [n, d+2] (PE; cols d..d+1 = c_b =
                                           1/sigmoid(bias_b); padded to an
                                           even moving dim for the ISA)
      rz      = 1 / dZ[:, d]               (DVE recip; = sigmoid(bias)/Z)
      y'      = y + rz * dZ[:, :d]         (DVE stt; t=0 only, f32)
      acc    += rz * dZ[:, :d]             (DVE stt; in-place per tile)
  The transposed-scores layout makes U^T directly usable as the delta-matmul
  lhsT (no per-tile attention transposes); only y' needs a PE transpose per
  bank (f32, group-batched) + one Act copy per chunk to feed step 1's
  scoresT.  Inputs load via rearranged 4-tile DMAs on the SP/Act hardware
  DGE queues; the output accumulates into 4 wide tiles stored with 4 DMAs.
  Banks are emitted interleaved (b0t0, b1t0, b0t1, b2t0, ...) so step-1's
  dependence on y1T never starves the PE.

  TRN2 constraints found the hard way: GPSIMD/Pool cannot access PSUM and
  does not implement TensorScalarPtr; AluOp divide is not in the hardware
  ISA (use DVE reciprocal + mult); matmul moving dim must be even; f32r
  transposes/memsets are invalid ISA (keep the transpose path plain f32).

Exact path (mask or non-tiny alpha): the original f32r kernel with the full
sigmoid(alpha * <y, delta> / D + bias) gate.
"""

import contextlib
import os

import numpy as np

import concourse.bass as bass
import concourse.mybir as mybir
import concourse.tile as tile
from concourse import bacc
from concourse.bass_utils import run_bass_kernel_spmd
from concourse.masks import make_identity

dt = mybir.dt
AL = mybir.AluOpType
AF = mybir.ActivationFunctionType

BS, B, M, D, N = 8, 4, 256, 256, 2048
P = 128
NT = N // P   # 16 row tiles of y
QB = 4        # n-tiles per gate batch (exact path; bounded by PSUM banks)
NQ = NT // QB
G = 4         # n-tiles per group (fast path)
NG = NT // G
N_STEPS = 2

f32 = dt.float32
bf16 = dt.bfloat16
f32r = dt.float32r


def _build_fast(tau, vcol, reps: int = 1):
    """vcol[b] = 1/sigmoid(bias_b): value of the augmented V column, so
    dividing delta by the Z column directly applies sigmoid(bias)/Z."""
    nc = bacc.Bacc(None, target_bir_lowering=False)
    seed_d = nc.dram_tensor("seed", [N, D], f32, kind="ExternalInput")
    emk_d = nc.dram_tensor("em_K", [B, M, D], f32, kind="ExternalInput")
    emv_d = nc.dram_tensor("em_V", [B, M, D], f32, kind="ExternalInput")
    out_d = nc.dram_tensor("out", [N, D], f32, kind="ExternalOutput")

    with tile.TileContext(nc) as tc:
        ctx = contextlib.ExitStack()
        with ctx:
            pool = lambda name, bufs, space="SBUF": ctx.enter_context(
                tc.tile_pool(name=name, bufs=bufs, space=space)
            )
            p_seed = pool("p_seed", NT // 4)
            p_acc = pool("p_acc", NT // 4)
            p_y1 = pool("p_y1", 3 * NT)
            p_sT = pool("p_sT", 2)
            p_y1T = pool("p_y1T", 4)
            p_kT = pool("p_kT", 2 * B)
            p_va = pool("p_va", 2 * B)
            p_stage = pool("p_stage", 2 * B)
            p_U = pool("p_U", 4)
            p_dcp = pool("p_dcp", 8)
            p_tiny = pool("p_tiny", 32)
            p_const = pool("p_const", 1)
            p_psT = pool("p_psT", 2, "PSUM")
            p_psd = pool("p_psd", 4, "PSUM")
            p_pt = pool("p_pt", 2, "PSUM")

            ident = p_const.tile([P, P], f32, name="ident")
            make_identity(nc, ident)

            dmaq = [nc.sync, nc.scalar]

            def dma(k, dst, src):
                dmaq[k % 2].dma_start(dst, src)

            def flex_copy(k, dst, src):
                # PSUM -> SBUF copies on Act (DVE is the bottleneck engine;
                # Pool cannot touch PSUM on TRN2)
                nc.scalar.copy(dst, src)

            for rep in range(reps):
                # ---- K/V first (gate the first matmuls), then seed ----
                kst = []
                vst = []
                for b in range(B):
                    k_st = p_stage.tile([P, 2 * D], f32, name="k_st")
                    dma(b, k_st, emk_d[b].rearrange("(h p) c -> p h c", h=2))
                    kst.append(k_st)
                    v_st = p_stage.tile([P, 2 * D], f32, name="v_st")
                    dma(b + 1, v_st, emv_d[b].rearrange("(h p) c -> p h c", h=2))
                    vst.append(v_st)
                seed4 = []
                for g4 in range(NT // 4):
                    s4 = p_seed.tile([P, 4 * D], f32, name="s4")
                    dma(
                        g4,
                        s4,
                        seed_d[g4 * 512 : (g4 + 1) * 512, :].rearrange(
                            "(b p) c -> p b c", b=4
                        ),
                    )
                    seed4.append(s4)
                seed_sb = [
                    seed4[i // 4][:, (i % 4) * D : (i % 4 + 1) * D] for i in range(NT)
                ]

                kT = []   # kT[b][c]: [P(d-chunk c), 256(m)] f32r
                va = []   # va[b][mh]: [P(m-chunk mh), 257] f32r (col 256 = vcol[b])
                for b in range(B):
                    kT_b = []
                    for c in range(2):
                        pt = p_pt.tile([P, 512], f32, name="pt")
                        nc.tensor.transpose(pt[:, 0:P], kst[b][:, c * P : (c + 1) * P], ident)
                        nc.tensor.transpose(
                            pt[:, P : 2 * P], kst[b][:, D + c * P : D + (c + 1) * P], ident
                        )
                        kc = p_kT.tile([P, M], f32r, name="kc")
                        flex_copy(c, kc, pt[:, 0 : 2 * P])
                        kT_b.append(kc)
                    kT.append(kT_b)
                    va_b = []
                    for mh in range(2):
                        vt = p_va.tile([P, D + 2], f32r, name="vt")
                        nc.gpsimd.tensor_copy(vt[:, 0:D], vst[b][:, mh * D : (mh + 1) * D])
                        nc.gpsimd.memset(vt[:, D : D + 2].bitcast(f32), vcol[b])
                        va_b.append(vt)
                    va.append(va_b)

                sT = [p_sT.tile([P, N], f32r, name="sT") for _ in range(2)]
                for q in range(NG):
                    pts = [p_pt.tile([P, 512], f32, name="pt") for _ in range(2)]
                    for j in range(G):
                        i = q * G + j
                        for c in range(2):
                            nc.tensor.transpose(
                                pts[c][:, j * P : (j + 1) * P],
                                seed_sb[i][:, c * P : (c + 1) * P],
                                ident,
                            )
                    for c in range(2):
                        flex_copy(q * 2 + c, sT[c][:, q * 512 : (q + 1) * 512], pts[c])

                # ---- main: banks interleaved so step-1 never starves PE ----
                acc4 = [None] * (NT // 4)
                acc = [None] * NT
                y1s = {}
                y1Ts = {}
                SCHED = [(0, 0), (1, 0), (0, 1), (2, 0), (1, 1), (3, 0), (2, 1), (3, 1)]
                for (b, t) in SCHED:
                    if t == 0:
                        yT = sT
                        y1_b = []
                        y1T_b = [p_y1T.tile([P, N], f32r, name="y1T") for _ in range(2)]
                    else:
                        yT = y1Ts[b]
                    for q in range(NG):
                        psT = [p_psT.tile([P, 512], f32, name="psT") for _ in range(2)]
                        for mh in range(2):
                            for c in range(2):
                                nc.tensor.matmul(
                                    psT[mh],
                                    kT[b][c][:, mh * P : (mh + 1) * P],
                                    yT[c][:, q * 512 : (q + 1) * 512],
                                    start=(c == 0), stop=(c == 1),
                                )
                        U = [p_U.tile([P, 512], f32r, name="U") for _ in range(2)]
                        for mh in range(2):
                            nc.scalar.activation(U[mh], psT[mh], AF.Exp, scale=1.0 / tau[b])
                        psd = []
                        for j in range(G):
                            ps = p_psd.tile([P, 512], f32, name="psd")
                            for mh in range(2):
                                nc.tensor.matmul(
                                    ps[:, 0 : D + 2],
                                    U[mh][:, j * P : (j + 1) * P],
                                    va[b][mh],
                                    start=(mh == 0), stop=(mh == 1),
                                )
                            psd.append(ps)
                        if t == 0:
                            pts = [p_pt.tile([P, 512], f32, name="pt") for _ in range(2)]
                        rzs = p_tiny.tile([P, G], f32, name="rzs")
                        for j in range(G):
                            i = q * G + j
                            ps = psd[j]
                            rz = rzs[:, j : j + 1]
                            if t == 0:
                                nc.vector.reciprocal(rz, ps[:, D : D + 1])
                                src, ueng = ps, nc.vector
                                y1_i = p_y1.tile([P, D], f32, name="y1_i")
                                ueng.scalar_tensor_tensor(
                                    y1_i, src[:, 0:D], rz, seed_sb[i], AL.mult, AL.add
                                )
                                if acc[i] is None:
                                    if acc4[i // 4] is None:
                                        acc4[i // 4] = p_acc.tile(
                                            [P, 4 * D], f32, name="a4"
                                        )
                                    a_i = acc4[i // 4][
                                        :, (i % 4) * D : (i % 4 + 1) * D
                                    ]
                                    # acc init = y1 - seed: SBUF-only, so the
                                    # otherwise-idle Pool engine can do it
                                    nc.gpsimd.tensor_tensor(
                                        a_i, y1_i, seed_sb[i], AL.subtract
                                    )
                                    acc[i] = a_i
                                else:
                                    ueng.scalar_tensor_tensor(
                                        acc[i], src[:, 0:D], rz, acc[i], AL.mult, AL.add
                                    )
                                for c in range(2):
                                    nc.tensor.transpose(
                                        pts[c][:, j * P : (j + 1) * P],
                                        y1_i[:, c * P : (c + 1) * P],
                                        ident,
                                    )
                                y1_b.append(y1_i)
                            else:
                                nc.vector.reciprocal(rz, ps[:, D : D + 1])
                                nc.vector.scalar_tensor_tensor(
                                    acc[i], ps[:, 0:D], rz, acc[i], AL.mult, AL.add
                                )
                        if t == 0:
                            for c in range(2):
                                flex_copy(
                                    q * 2 + c + 1,
                                    y1T_b[c][:, q * 512 : (q + 1) * 512],
                                    pts[c],
                                )
                    if t == 0:
                        y1s[b] = y1_b
                        y1Ts[b] = y1T_b

                for g4 in range(NT // 4):
                    (nc.sync if g4 % 2 == 0 else nc.scalar).dma_start(
                        out_d[g4 * 512 : (g4 + 1) * 512, :].rearrange(
                            "(b p) c -> p b c", b=4
                        ),
                        acc4[g4],
                    )

    nc.compile()
    return nc


def _build_exact(variant: str, tau, alpha, bias, use_mask: bool, reps: int = 1):
    DT = dt.bfloat16 if variant == "bf16" else f32
    DTmm = dt.float32r if variant == "f32r" else DT
    xbar = variant == "bf16"

    nc = bacc.Bacc(None, target_bir_lowering=False)
    seed_d = nc.dram_tensor("seed", [N, D], f32, kind="ExternalInput")
    emk_d = nc.dram_tensor("em_K", [B, M, D], f32, kind="ExternalInput")
    emv_d = nc.dram_tensor("em_V", [B, M, D], f32, kind="ExternalInput")
    out_d = nc.dram_tensor("out", [N, D], f32, kind="ExternalOutput")
    if use_mask:
        msk_d = nc.dram_tensor("mask", [B, P, M], f32, kind="ExternalInput")

    with tile.TileContext(nc) as tc:
        ctx = contextlib.ExitStack()
        with ctx:
            pool = lambda name, bufs, space="SBUF": ctx.enter_context(
                tc.tile_pool(name=name, bufs=bufs, space=space)
            )
            p_s = pool("p_s", NT)
            p_sdt = pool("p_sdt", NT) if xbar else None
            p_sT = pool("p_sT", NT)
            p_k = pool("p_k", B)
            p_v = pool("p_v", B)
            p_acc = pool("p_acc", NT)
            p_y1 = pool("p_y1", 2 * NT)
            p_y1T = pool("p_y1T", 2 * NT)
            p_U = pool("p_U", 6)
            p_uT = pool("p_uT", 6)
            p_stage = pool("p_stage", 4)
            p_scr = pool("p_scr", 4)
            p_tiny = pool("p_tiny", 32)
            p_ps = pool("p_ps", 8 if xbar else 6, space="PSUM")
            p_pt = None if xbar else pool("p_pt", 2, space="PSUM")
            p_const = pool("p_const", 1)
            p_msk = pool("p_msk", B) if use_mask else None

            ident = None
            if not xbar:
                ident = p_const.tile([P, P], f32, name="ident")
                make_identity(nc, ident)

            def transp_to(dst, srcs):
                if xbar:
                    for src, c in srcs:
                        nc.sync.dma_start(dst[:, c : c + P], src, transpose=True)
                else:
                    w = max(c for _, c in srcs) + P
                    pt = p_pt.tile([P, 512], f32, name="pt")
                    for src, c in srcs:
                        nc.tensor.transpose(pt[:, c : c + P], src, ident)
                    nc.vector.tensor_copy(dst[:, 0:w], pt[:, 0:w])

            for rep in range(reps):
                sb_s = []
                s_src = []
                for i in range(NT):
                    s_i = p_s.tile([P, D], f32, name="s_i")
                    nc.gpsimd.dma_start(s_i, seed_d[i * P : (i + 1) * P, :])
                    sb_s.append(s_i)
                    if xbar:
                        sdt_i = p_sdt.tile([P, D], DT, name="sdt_i")
                        nc.gpsimd.dma_start(sdt_i, seed_d[i * P : (i + 1) * P, :])
                        s_src.append(sdt_i)
                    else:
                        s_src.append(s_i)

                msk = []
                if use_mask:
                    for b in range(B):
                        m_b = p_msk.tile([P, M], f32, name="m_b")
                        nc.gpsimd.dma_start(m_b, msk_d[b])
                        msk.append(m_b)

                v = []
                kT = []
                for b in range(B):
                    v_b = p_v.tile([P, 2 * D], DTmm, name="v_b")
                    for mh in range(2):
                        if DTmm == dt.float32r:
                            ev_t = p_stage.tile([P, D], f32, name="ev_t")
                            nc.gpsimd.dma_start(
                                ev_t, emv_d[b, mh * P : (mh + 1) * P, :]
                            )
                            nc.vector.tensor_copy(v_b[:, mh * D : (mh + 1) * D], ev_t)
                        else:
                            nc.gpsimd.dma_start(
                                v_b[:, mh * D : (mh + 1) * D],
                                emv_d[b, mh * P : (mh + 1) * P, :],
                            )
                    v.append(v_b)
                    ek = []
                    for mt in range(2):
                        ek_t = p_stage.tile([P, D], DT, name="ek_t")
                        nc.gpsimd.dma_start(ek_t, emk_d[b, mt * P : (mt + 1) * P, :])
                        ek.append(ek_t)
                    kT_b = p_k.tile([P, 2 * M], DTmm, name="kT_b")
                    transp_to(
                        kT_b,
                        [
                            (ek[0][:, 0:P], 0),
                            (ek[0][:, P : 2 * P], 2 * P),
                            (ek[1][:, 0:P], P),
                            (ek[1][:, P : 2 * P], 3 * P),
                        ],
                    )
                    kT.append(kT_b)

                sT = []
                for i in range(NT):
                    sT_i = p_sT.tile([P, 2 * P], DTmm, name="sT_i")
                    transp_to(sT_i, [(s_src[i][:, 0:P], 0), (s_src[i][:, P : 2 * P], P)])
                    sT.append(sT_i)

                acc = [None] * NT

                y1_cur, y1T_cur = None, None
                for b in range(B):
                    for t in range(N_STEPS):
                        lhsT = sT if t == 0 else y1T_cur
                        yprev = sb_s if t == 0 else y1_cur
                        y1_new, y1T_new = [], []
                        for q in range(NQ):
                            zs = p_tiny.tile([P, QB], f32, name="zs")
                            dots = p_tiny.tile([P, QB], f32, name="dots")
                            pss = []
                            for j in range(QB):
                                i = q * QB + j
                                ps = p_ps.tile([P, 512], f32, name="ps")
                                pss.append(ps)
                                nc.tensor.matmul(
                                    ps[:, 0:M], lhsT[i][:, 0:P], kT[b][:, 0:M],
                                    start=True, stop=False,
                                )
                                nc.tensor.matmul(
                                    ps[:, 0:M], lhsT[i][:, P : 2 * P], kT[b][:, M : 2 * M],
                                    start=False, stop=True,
                                )
                                U = p_U.tile([P, M], DT, name="U")
                                if use_mask:
                                    nc.scalar.activation(U, ps[:, 0:M], AF.Exp, scale=1.0 / tau[b])
                                    nc.vector.tensor_tensor(U, U, msk[b], AL.mult)
                                    nc.vector.tensor_reduce(
                                        zs[:, j : j + 1], U, mybir.AxisListType.X, AL.add
                                    )
                                else:
                                    nc.scalar.activation(
                                        U, ps[:, 0:M], AF.Exp,
                                        scale=1.0 / tau[b], accum_out=zs[:, j : j + 1],
                                    )
                                uT = p_uT.tile([P, 2 * P], DTmm, name="uT")
                                transp_to(uT, [(U[:, 0:P], 0), (U[:, P : 2 * P], P)])
                                nc.tensor.matmul(
                                    ps[:, M : M + D], uT[:, 0:P], v[b][:, 0:D],
                                    start=True, stop=False,
                                )
                                nc.tensor.matmul(
                                    ps[:, M : M + D], uT[:, P : 2 * P], v[b][:, D : 2 * D],
                                    start=False, stop=True,
                                )
                                scr = p_scr.tile([P, D], f32, name="scr")
                                nc.vector.scalar_tensor_tensor(
                                    scr, ps[:, M : M + D], 1.0, yprev[i],
                                    AL.bypass, AL.mult, accum_out=dots[:, j : j + 1],
                                )
                            rzs = p_tiny.tile([P, QB], f32, name="rzs")
                            nc.vector.reciprocal(rzs, zs)
                            dn = p_tiny.tile([P, QB], f32, name="dn")
                            nc.vector.tensor_tensor(dn, dots, rzs, AL.mult)
                            e1 = p_tiny.tile([P, QB], f32, name="e1")
                            nc.scalar.activation(
                                e1, dn, AF.Exp, scale=-alpha[b] / D, bias=-bias[b]
                            )
                            ge = p_tiny.tile([P, QB], f32, name="ge")
                            nc.vector.tensor_scalar_add(ge, e1, 1.0)
                            gate = p_tiny.tile([P, QB], f32, name="gate")
                            nc.vector.reciprocal(gate, ge)
                            g = p_tiny.tile([P, QB], f32, name="g")
                            nc.vector.tensor_tensor(g, gate, rzs, AL.mult)
                            for j in range(QB):
                                i = q * QB + j
                                ps = pss[j]
                                gj = g[:, j : j + 1]
                                if b == 0 and t == 0:
                                    a_i = p_acc.tile([P, D], f32, name="a_i")
                                    nc.vector.tensor_scalar(
                                        a_i, ps[:, M : M + D], gj, None, AL.mult
                                    )
                                    acc[i] = a_i
                                else:
                                    nc.vector.scalar_tensor_tensor(
                                        acc[i], ps[:, M : M + D], gj, acc[i], AL.mult, AL.add
                                    )
                                if t == 0:
                                    y1_i = p_y1.tile([P, D], DT, name="y1_i")
                                    nc.vector.scalar_tensor_tensor(
                                        y1_i, ps[:, M : M + D], gj, yprev[i], AL.mult, AL.add
                                    )
                                    y1T_i = p_y1T.tile([P, 2 * P], DTmm, name="y1T_i")
                                    transp_to(
                                        y1T_i, [(y1_i[:, 0:P], 0), (y1_i[:, P : 2 * P], P)]
                                    )
                                    y1_new.append(y1_i)
                                    y1T_new.append(y1T_i)
                        if t == 0:
                            y1_cur, y1T_cur = y1_new, y1T_new

                for i in range(NT):
                    nc.gpsimd.dma_start(out_d[i * P : (i + 1) * P, :], acc[i])

    nc.compile()
    return nc


def _build(variant: str, tau, alpha, bias, use_mask: bool, reps: int = 1):
    if variant == "fast":
        sig = [1.0 / (1.0 + float(np.exp(-b))) for b in bias]
        vcol = [1.0 / s for s in sig]
        return _build_fast(tau, vcol, reps=reps)
    return _build_exact(variant, tau, alpha, bias, use_mask, reps=reps)


def kernel(**inputs):
    seed = np.ascontiguousarray(np.asarray(inputs["seed"], dtype=np.float32))
    em_K = np.ascontiguousarray(np.asarray(inputs["em_K"], dtype=np.float32))
    em_V = np.ascontiguousarray(np.asarray(inputs["em_V"], dtype=np.float32))
    em_S = np.asarray(inputs["em_S"], dtype=np.float32)
    gate_alpha = np.asarray(inputs["gate_alpha"], dtype=np.float32)
    gate_bias = np.asarray(inputs["gate_bias"], dtype=np.float32)
    raw_tau = np.asarray(inputs["raw_tau"], dtype=np.float32)

    tau = [float(np.log1p(np.exp(raw_tau[b])) + 0.1) for b in range(B)]
    alpha = [float(gate_alpha[b]) for b in range(B)]
    bias = [float(gate_bias[b]) for b in range(B)]
    use_mask = bool((em_S <= 0).any())

    variant = os.environ.get("EM_VARIANT", "")
    if not variant:
        # the constant-gate fast path is valid when the sigmoid barely moves
        fast_ok = (not use_mask) and max(abs(a) for a in alpha) <= 0.05
        variant = "fast" if fast_ok else "f32r"

    nc = _build(variant, tau, alpha, bias, use_mask)

    in_maps = []
    for c in range(BS):
        m = {"seed": seed[c], "em_K": em_K[c], "em_V": em_V[c]}
        if use_mask and variant != "fast":
            mask = (em_S[c] > 0).astype(np.float32)  # [B, M]
            m["mask"] = np.ascontiguousarray(
                np.broadcast_to(mask[:, None, :], (B, P, M))
            )
        in_maps.append(m)

    res = run_bass_kernel_spmd(nc, in_maps, core_ids=list(range(BS)))
    out = np.stack([res.results[c]["out"] for c in range(BS)], axis=0)
    return out.astype(np.float32)
